# revision 63
# baseline (speedup 1.0000x reference)
"""Trainium2 Bass kernel for nn_EncoderBlock (dual self-attention + BN + FFN + BN).

Sharding: data-parallel over batch (16 batches -> 2 per core on 8 cores).

v2: the attention block runs in fp8e4m3 with DoubleRow matmuls (4x PE rate on
the qkv/out projections and AV), validated to ~8e-4 end-to-end rel err.
Scores stay bf16 (64-deep contraction can't DoubleRow). exp outputs fp8
directly from ACT with the softmax /8 range shift folded into the exp bias;
the ones-column (value 1/64) inside V yields denominators from the AV matmul.
K/Q/O biases are per-partition scalars in the evacuation ops (K's provably
cancels in softmax but is applied anyway); V's bias rides the existing
psum->v8 add. The out-projection evacuation is a fused scalar_tensor_tensor
(psum + bias + residual) whose accum_out doubles as the BN1 channel sums;
sumsq comes from a square pass split across DVE/ACT. BN1's scale is folded
into the F1 weights (in-place) so the FFN starts right after the AllReduce;
BN1's shift becomes an F1 bias correction via a tiny matvec. BN2 stats use
the same accum trick; the final normalize+store is pipelined per 512-column
slice across DVE/ACT/Pool with immediate per-slice DMA.
BatchNorm batch stats use a 4KB AllReduce across the 8 cores (twice).
"""

import numpy as np
import concourse.bass as bass
import concourse.bacc as bacc
import concourse.tile as tile
from concourse import mybir
from concourse.bass_utils import run_bass_kernel_spmd

dt = mybir.dt
F32 = dt.float32
F32R = dt.float32r
BF16 = dt.bfloat16
F8 = dt.float8e4
AF = mybir.ActivationFunctionType
OP = mybir.AluOpType
DR = mybir.MatmulPerfMode.DoubleRow

N_CORES = 8
B, N, E, H, DK = 16, 1024, 512, 8, 64
NR, NT = 256, 768          # robot / task sequence lengths
BL = B // N_CORES          # local batches per core
TOK = BL * N               # local tokens per core
EC = E // 128              # channel chunks of 128
N_GLOBAL = B * N           # BN stat count
EPS = 1e-5
LN8 = 2.0794415416798357   # exp range shift: ex = exp(s)/8
IVS = 1.0 / 64.0           # ones-column value (denominator scale)

W8_NAMES = ["rq", "rk", "rv", "ro", "tq", "tk", "tv", "to"]
ALL_B = W8_NAMES + ["f1", "f2", "bn1_g", "bn1_b", "bn2_g", "bn2_b"]


def _bank_slices(base, length, maxlen=512):
    """Split [base, base+length) into pieces (<=maxlen) that never cross a
    512-col PSUM bank boundary."""
    out = []
    cur = base
    end = base + length
    while cur < end:
        nb = (cur // 512 + 1) * 512
        fl = min(end, min(nb, cur + maxlen)) - cur
        out.append((cur - base, fl))
        cur += fl
    return out


def build(for_timing=False):
    nc = bacc.Bacc("TRN2", target_bir_lowering=False, debug=False,
                   num_devices=N_CORES)

    xT_d = nc.dram_tensor("xT", [E, TOK], F32, kind="ExternalInput")
    x8r_d = nc.dram_tensor("x8r", [128, EC * 2 * NR], F8, kind="ExternalInput")
    x8t_d = nc.dram_tensor("x8t", [128, EC * 2 * NT], F8, kind="ExternalInput")
    w8_d = {n: nc.dram_tensor(f"{n}_w8", [128, 4 * E], F8, kind="ExternalInput")
            for n in W8_NAMES}
    f_d = {"f1": nc.dram_tensor("f1_wT", [E, E], F32, kind="ExternalInput"),
           "f2": nc.dram_tensor("f2_w8", [128, 4 * E], F8, kind="ExternalInput")}
    bpk_d = nc.dram_tensor("bpk", [128, len(ALL_B) * EC], F32,
                           kind="ExternalInput")
    vrep_d = {n: nc.dram_tensor(f"{n}_brep", [128, E], F32, kind="ExternalInput")
              for n in ["rv", "tv"]}
    sel2_d = nc.dram_tensor("sel2", [98, 128], F32, kind="ExternalInput")
    ones1_d = nc.dram_tensor("ones1", [1, 128], F32, kind="ExternalInput")
    yT_d = nc.dram_tensor("yT", [E, TOK], F32, kind="ExternalOutput")

    from contextlib import ExitStack
    with tile.TileContext(nc) as tc, ExitStack() as es:
        const = es.enter_context(tc.tile_pool(name="const", bufs=1))
        wpool = es.enter_context(tc.tile_pool(name="w", bufs=1))
        act = es.enter_context(tc.tile_pool(name="act", bufs=1))
        attn = es.enter_context(tc.tile_pool(name="attn", bufs=2))
        expp = es.enter_context(tc.tile_pool(name="expp", bufs=2))
        small = es.enter_context(tc.tile_pool(name="small", bufs=2))
        dram = es.enter_context(tc.tile_pool(name="dram", bufs=1, space="DRAM"))
        ps_big = es.enter_context(tc.tile_pool(name="ps_big", bufs=2, space="PSUM"))
        ps_av = es.enter_context(tc.tile_pool(name="ps_av", bufs=2, space="PSUM"))
        _body(nc, const, wpool, act, attn, expp, small, dram, ps_big, ps_av,
              xT_d, x8r_d, x8t_d, w8_d, f_d, bpk_d, vrep_d, sel2_d, ones1_d,
              yT_d, for_timing)
    nc.finalize()
    return nc


class _Ctx:
    pass


def _body(nc, const, wpool, act, attn, expp, small, dram, ps_big, ps_av,
          xT_d, x8r_d, x8t_d, w8_d, f_d, bpk_d, vrep_d, sel2_d, ones1_d,
          yT_d, for_timing):
    # ---------- constants / inputs resident in SBUF ----------
    # DMA emission order == issue order on the sync queue: the first
    # projection needs rq/rk weights + robot x8 columns; bulk (xT, task x8,
    # FFN weights) trickles in behind.
    w8 = {}
    t = wpool.tile([128, 2, 2, E], F8, tag="w8rq", name="w8rq")
    nc.sync.dma_start(out=t[:], in_=w8_d["rq"].ap())
    w8["rq"] = t
    x8r = const.tile([128, EC, 2, NR], F8, tag="x8r", name="x8r")
    x8t = const.tile([128, EC, 2, NT], F8, tag="x8t", name="x8t")
    nc.sync.dma_start(out=x8r[:], in_=x8r_d.ap())
    t = wpool.tile([128, 2, 2, E], F8, tag="w8rk", name="w8rk")
    nc.sync.dma_start(out=t[:], in_=w8_d["rk"].ap())
    w8["rk"] = t
    bpk = const.tile([128, len(ALL_B) * EC], F32, tag="bpk", name="bpk")
    nc.sync.dma_start(out=bpk[:], in_=bpk_d.ap())
    bias = {n: bpk[:, i * EC:(i + 1) * EC] for i, n in enumerate(ALL_B)}
    for n in ["rv", "ro"]:
        t = wpool.tile([128, 2, 2, E], F8, tag=f"w8{n}", name=f"w8{n}")
        nc.sync.dma_start(out=t[:], in_=w8_d[n].ap())
        w8[n] = t
    vrep = {}
    t = const.tile([128, E], F32R, tag="vr_rv", name="vr_rv")
    nc.sync.dma_start(out=t[:], in_=vrep_d["rv"].ap().bitcast(F32R))
    vrep["rv"] = t
    sel2 = const.tile([98, 128], F32R, tag="sel2", name="sel2")
    nc.sync.dma_start(out=sel2[:], in_=sel2_d.ap().bitcast(F32R))
    ones1 = const.tile([1, 128], F32R, tag="ones1", name="ones1")
    nc.sync.dma_start(out=ones1[:], in_=ones1_d.ap().bitcast(F32R))
    # Bulk loads ride the Pool SWDGE train (no HWDGE contention with the
    # latency-critical z8/rows transfers on sync), ordered by need time:
    # xT robot (outproj R0 ~18us) -> task x8/weights (T0 proj ~30us) ->
    # xT task (outproj T0 ~110us) -> FFN weights (BN1 ~170us).
    xT = [const.tile([128, TOK], F32R, tag=f"xT{k}", name=f"xT{k}")
          for k in range(EC)]
    for k in range(EC):
        nc.sync.dma_start(out=xT[k][:, 0:NR],
                          in_=xT_d.ap()[k * 128:(k + 1) * 128, 0:NR].bitcast(F32R))
        nc.sync.dma_start(out=xT[k][:, N:N + NR],
                          in_=xT_d.ap()[k * 128:(k + 1) * 128, N:N + NR].bitcast(F32R))
    x8td = x8t_d.ap().rearrange("p (k b t) -> p k b t", k=EC, b=2)
    nc.gpsimd.dma_start(out=x8t[:, :, 0, :], in_=x8td[:, :, 0, :])
    nc.gpsimd.dma_start(out=x8t[:, :, 1, :], in_=x8td[:, :, 1, :])
    for n in ["tq", "tk", "tv", "to"]:
        t = wpool.tile([128, 2, 2, E], F8, tag=f"w8{n}", name=f"w8{n}")
        nc.gpsimd.dma_start(out=t[:], in_=w8_d[n].ap())
        w8[n] = t
    t = const.tile([128, E], F32R, tag="vr_tv", name="vr_tv")
    nc.gpsimd.dma_start(out=t[:], in_=vrep_d["tv"].ap().bitcast(F32R))
    vrep["tv"] = t
    for k in range(EC):
        nc.sync.dma_start(out=xT[k][:, NR:N],
                          in_=xT_d.ap()[k * 128:(k + 1) * 128, NR:N].bitcast(F32R))
        nc.sync.dma_start(out=xT[k][:, N + NR:],
                          in_=xT_d.ap()[k * 128:(k + 1) * 128, N + NR:].bitcast(F32R))
    # FFN weights (f1 f32r: folded in place later; f2 bf16 to match bf16 h1)
    f1 = [wpool.tile([128, E], F32R, tag=f"f1_{k}", name=f"f1_{k}")
          for k in range(EC)]
    f28 = wpool.tile([128, 2, 2, E], F8, tag="f28", name="f28")
    for k in range(EC):
        nc.gpsimd.dma_start(out=f1[k][:],
                            in_=f_d["f1"].ap()[k * 128:(k + 1) * 128, :].bitcast(F32R))
    nc.gpsimd.dma_start(out=f28[:], in_=f_d["f2"].ap())

    # prefetch the exp ACT table set while input DMAs are in flight
    warm = const.tile([1, 1], F32, tag="warm", name="warm")
    nc.vector.memset(warm[:], 0.0)
    nc.scalar.activation(out=warm[:], in_=warm[:], func=AF.Exp, scale=1.0)
    negln8 = const.tile([128, 1], F32, tag="negln8", name="negln8")
    nc.gpsimd.memset(negln8[:], -LN8)
    epst = const.tile([128, 1], F32, tag="epst", name="epst")
    nc.gpsimd.memset(epst[:], EPS)


    # h-tilde (pre-BN1 attention output) accumulated across parts/batches,
    # with per-(m, batch-part) channel sums / sumsq for BN1
    ht = [act.tile([128, TOK], F32R, tag=f"ht{k}", name=f"ht{k}")
          for k in range(EC)]
    acc1 = small.tile([128, EC, 4], F32, tag="acc1", name="acc1", bufs=1)
    sq1 = small.tile([128, EC, 4], F32, tag="sq1", name="sq1", bufs=1)

    # ---------- attention (fine-grained interleaved emission) ----------
    # Per part, per m-chunk: project q/k chunk m, interleave the PREVIOUS
    # part's output-projection chunk m, then run head pair m (scores -> exp
    # -> AV -> evac) and its denominator broadcast + z8 scale. The exp stream
    # on ACT paces everything; PE/DVE work rides underneath it. Robot parts
    # are DVE-bound instead, so their k/zu evacuations go to ACT.
    def make_state(P):
        st = _Ctx()
        st.qT = [attn.tile([128, NT], BF16, tag=f"qT{m}", name=f"qT{m}")
                 for m in range(EC)]
        st.kT = [attn.tile([128, NT], BF16, tag=f"kT{m}", name=f"kT{m}")
                 for m in range(EC)]
        st.v8 = attn.tile([128, 6, H, DK + 2], F8, tag="v8", name="v8")
        st.z8 = attn.tile([128, 4, NT], F8, tag="z8", name="z8")
        st.rows = small.tile([98, NT], F8, tag="rows", name="rows")
        st.rinv = small.tile([98, NT], F32R, tag="rinv", name="rinv")
        if P.merged:
            st.x8p = [x8r[:, :, b, :] for b in range(2)]
        else:
            st.x8p = [x8t[:, :, P.b, :]]
        with nc.allow_low_precision(reason="fp8 attention"):
            nc.vector.memset(st.v8[:, :, :, DK:DK + 1], IVS)
            nc.vector.memset(st.v8[:, :, :, DK + 1:DK + 2], 0.0)
        return st

    def emit_qk(P, st, m):
        ps = ps_big.tile([128, 2, NT], F32, tag="sc", name="psq")
        for r, (wt, o_t, bname) in enumerate(
                [(P.wq, st.qT, P.wn[0]), (P.wk, st.kT, P.wn[1])]):
            for b in range(P.nb):
                for off, fl in _bank_slices(b * P.np, P.np, 256):
                    for g in range(2):
                        nc.tensor.matmul(
                            ps[:, r, b * P.np + off:b * P.np + off + fl],
                            wt[:, g, :, m * 128:(m + 1) * 128],
                            st.x8p[b][:, 2 * g:2 * g + 2, off:off + fl],
                            start=(g == 0), stop=(g == 1), perf_mode=DR)
            with nc.allow_low_precision(reason="bf16 qk"):
                if P.merged and r == 1:
                    nc.scalar.activation(
                        out=o_t[m][:, 0:P.w], in_=ps[:, r, 0:P.w],
                        func=AF.Identity, bias=bias[bname][:, m:m + 1],
                        scale=1.0)
                else:
                    nc.vector.tensor_scalar(
                        out=o_t[m][:, 0:P.w], in0=ps[:, r, 0:P.w],
                        scalar1=bias[bname][:, m:m + 1], scalar2=None,
                        op0=OP.add)

    def emit_v(P, st, b, t):
        ps = ps_big.tile([128, 2, NT], F32, tag="sc", name="psv")
        for j0 in (0, 256):
            for g in range(2):
                nc.tensor.matmul(
                    ps[:, 0, j0:j0 + 256],
                    st.x8p[b][:, 2 * g:2 * g + 2, t * 128:(t + 1) * 128],
                    P.wv[:, g, :, j0:j0 + 256],
                    start=(g == 0), stop=(g == 1), perf_mode=DR)
            # V bias via a K=1 ones-row matmul so the evac is a pure copy
            # that can ride the otherwise-idle ACT slots
            nc.tensor.matmul(
                ps[:, 0, j0:j0 + 256], ones1[:],
                vrep[P.wn[2]][0:1, j0:j0 + 256],
                start=False, stop=False, skip_group_check=True)
        with nc.allow_low_precision(reason="fp8 v"):
            nc.scalar.activation(
                out=st.v8[:, b * P.nk + t, :, 0:DK],
                in_=ps[:, 0, 0:E].rearrange("p (h d) -> p h d", h=H),
                func=AF.Copy)

    def emit_scores(P, st, pair, j):
        exs = []
        for g in range(P.nk // 2):
            sc = ps_big.tile([128, 2, NT], F32, tag="sc", name="sc")
            for b in range(P.nb):
                qh = st.qT[pair][j * 64:j * 64 + 64,
                                 b * P.np:(b + 1) * P.np]
                for j2 in range(2):
                    kc = 2 * g + j2
                    kh = st.kT[pair][j * 64:j * 64 + 64,
                                     b * P.np + kc * 128:
                                     b * P.np + (kc + 1) * 128]
                    for off, fl in _bank_slices(j2 * NT + b * P.np, P.np):
                        nc.tensor.matmul(
                            sc[:, j2, b * P.np + off:b * P.np + off + fl],
                            kh, qh[:, off:off + fl],
                            start=True, stop=True)
            ex = expp.tile([128, 2, NT], F8, tag="exp", name="exp", bufs=9)
            with nc.allow_low_precision(reason="fp8 exp"):
                nc.scalar.activation(
                    out=ex[:, :, 0:P.w], in_=sc[:, :, 0:P.w],
                    func=AF.Exp, scale=0.125, bias=negln8[:])
            exs.append(ex)
        return exs

    def emit_avs(P, st, pair, j, exs, zst_box):
        h = 2 * pair + j
        zu = ps_av.tile([66, NT], F32, tag="av", name="av", bufs=1)
        for b in range(P.nb):
            base = b * P.np
            for off, fl in _bank_slices(base, P.np, 256):
                for g in range(P.nk // 2):
                    nc.tensor.matmul(
                        zu[:, base + off:base + off + fl],
                        st.v8[:, b * P.nk + 2 * g:b * P.nk + 2 * g + 2, h, :],
                        exs[g][:, :, base + off:base + off + fl],
                        start=(g == 0), stop=(g == P.nk // 2 - 1),
                        perf_mode=DR)
        if j == 0:
            zst_box[0] = expp.tile([65, 2, NT], F8, tag="zst", name="zst",
                                   bufs=3)
        zst = zst_box[0]
        with nc.allow_low_precision(reason="fp8 z"):
            if P.merged:
                nc.scalar.activation(out=zst[:, j, 0:P.w],
                                     in_=zu[0:65, 0:P.w], func=AF.Copy)
            else:
                nc.vector.tensor_copy(out=zst[:, j, 0:P.w],
                                      in_=zu[0:65, 0:P.w])
        nc.sync.dma_start(out=st.z8[64 * j:64 * (j + 1), pair, 0:P.w],
                          in_=zst[0:64, j, 0:P.w])
        if j == 1:
            nc.sync.dma_start(out=st.rows[32 * pair:32 * pair + 2, 0:P.w],
                              in_=zst[64:65, :, 0:P.w])
            with nc.allow_low_precision(reason="f32r feeds f32r mm"):
                nc.vector.reciprocal(
                    out=st.rinv[32 * pair:32 * pair + 2, 0:P.w],
                    in_=st.rows[32 * pair:32 * pair + 2, 0:P.w])

    def emit_pair_denom(P, st, pair):
        rinv = st.rinv[32 * pair:32 * pair + 2, 0:P.w]
        rep = ps_big.tile([128, 2, NT], F32, tag="sc", name="rep")
        tp = (96, 0) if pair == 3 else None
        for off, fl in _bank_slices(0, P.w):
            nc.tensor.matmul(rep[:, 0, off:off + fl],
                             sel2[32 * pair:32 * pair + 2, :],
                             rinv[:, off:off + fl], start=True, stop=True,
                             tile_position=tp)
        with nc.allow_low_precision(reason="fp8 z scale"):
            nc.vector.tensor_tensor(out=st.z8[:, pair, 0:P.w],
                                    in0=st.z8[:, pair, 0:P.w],
                                    in1=rep[:, 0, 0:P.w], op=OP.mult)

    def emit_outproj(P, st, m):
        ps = ps_big.tile([128, 2, NT], F32, tag="sc", name="pso")
        for b in range(P.nb):
            base = b * P.np
            for off, fl in _bank_slices(base, P.np, 256):
                for g in range(2):
                    nc.tensor.matmul(
                        ps[:, 0, base + off:base + off + fl],
                        P.wo[:, g, :, m * 128:(m + 1) * 128],
                        st.z8[:, 2 * g:2 * g + 2, base + off:base + off + fl],
                        start=(g == 0), stop=(g == 1), perf_mode=DR)
        if P.merged:
            dst = ht[m][:].rearrange("p (b n) -> p b n", b=2)[:, :, 0:NR]
            res = xT[m][:].rearrange("p (b n) -> p b n", b=2)[:, :, 0:NR]
            src_ps = ps[:, 0, 0:P.w].rearrange("p (b n) -> p b n", b=2)
        else:
            dst = ht[m][:, P.tok0:P.tok0 + P.np]
            res = xT[m][:, P.tok0:P.tok0 + P.np]
            src_ps = ps[:, 0, 0:P.np]
        with nc.allow_low_precision(reason="f32r ht"):
            nc.vector.scalar_tensor_tensor(
                out=dst, in0=src_ps,
                scalar=bias[P.wn[3]][:, m:m + 1],
                in1=res,
                op0=OP.add, op1=OP.add,
                accum_out=acc1[:, m, P.bp_idx:P.bp_idx + 1])
        dv = dst.bitcast(F32)
        if P.merged:
            def scr_out(scr):
                return scr[:, 0:P.w].rearrange("p (b n) -> p b n", b=2)
        else:
            def scr_out(scr):
                return scr[:, 0:P.np]
        if not P.sq_act:
            scr = small.tile([128, NT], F32, tag="sqd", name="sqd", bufs=2)
            nc.vector.scalar_tensor_tensor(
                out=scr_out(scr), in0=dv, scalar=1.0, in1=dv,
                op0=OP.mult, op1=OP.mult,
                accum_out=sq1[:, m, P.bp_idx:P.bp_idx + 1])
        else:
            scr = small.tile([128, NT], F32, tag="sqa", name="sqa", bufs=2)
            nc.scalar.activation(
                out=scr_out(scr), in_=dv, func=AF.Square,
                accum_out=sq1[:, m, P.bp_idx:P.bp_idx + 1])

    parts = []
    for bp_idx, (part, b) in enumerate([(0, 0), (1, 0), (1, 1)]):
        P = _Ctx()
        P.part = part
        P.b = b
        P.bp_idx = bp_idx
        P.merged = (part == 0)
        P.nb = 2 if P.merged else 1
        P.wn = ["rq", "rk", "rv", "ro"] if part == 0 else ["tq", "tk", "tv", "to"]
        P.np = NR if part == 0 else NT
        P.w = P.nb * P.np
        P.nk = P.np // 128
        P.tok0 = b * N + NR
        P.wq, P.wk, P.wv, P.wo = (w8[P.wn[0]], w8[P.wn[1]], w8[P.wn[2]],
                                  w8[P.wn[3]])
        parts.append(P)

    def v_sched(P):
        bts = [(b, t) for b in range(P.nb) for t in range(P.nk)]
        out = [[], [], [], []]
        for idx, bt in enumerate(bts):
            out[min(3, idx * 4 // len(bts))].append(bt)
        return out

    sts = {0: make_state(parts[0])}
    P0 = parts[0]
    for m in range(EC):
        emit_qk(P0, sts[0], m)
        for b, t in v_sched(P0)[m]:
            emit_v(P0, sts[0], b, t)
    pend = [None]

    def flush():
        if pend[0] is not None:
            pend[0]()
            pend[0] = None

    prev = None
    for i, P in enumerate(parts):
        st = sts[i]
        P.sq_act = (i == 2)   # mid-task ACT is the pacer; only trailing has slack
        nxt = parts[i + 1] if i + 1 < len(parts) else None
        if nxt is not None:
            sts[i + 1] = make_state(nxt)
        zst_box = [None]
        for pair in range(4):
            for j in (0, 1):
                exs = emit_scores(P, st, pair, j)
                flush()

                def mk(P=P, st=st, pair=pair, j=j, exs=exs, zb=zst_box,
                       prev=prev, nxt=nxt, i=i):
                    def run():
                        emit_avs(P, st, pair, j, exs, zb)
                        if j == 0:
                            # mid-pair: previous part's output projection
                            # (its final denominator first, once)
                            if prev is not None:
                                if pair == 0:
                                    emit_pair_denom(prev[0], prev[1], 3)
                                emit_outproj(prev[0], prev[1], pair)
                        else:
                            # pair boundary: own denominators one pair late,
                            # next part's projections
                            if pair >= 1:
                                emit_pair_denom(P, st, pair - 1)
                            if nxt is not None:
                                emit_qk(nxt, sts[i + 1], pair)
                                for b, t in v_sched(nxt)[pair]:
                                    emit_v(nxt, sts[i + 1], b, t)
                    return run
                pend[0] = mk()
        prev = (P, st)
    flush()
    emit_pair_denom(prev[0], prev[1], 3)
    for m in range(EC):
        emit_outproj(prev[0], prev[1], m)

    # all exps done: swap the ACT table set to sqrt ahead of BN1
    warm2 = const.tile([1, 1], F32, tag="warm", name="warm2")
    nc.vector.memset(warm2[:], 1.0)
    nc.scalar.activation(out=warm2[:], in_=warm2[:], func=AF.Sqrt, scale=1.0)

    # ---------- BN1 (sums -> AllReduce -> params; fold into F1) ----------
    s1, t1 = _bn_params(nc, small, dram, acc1[:, :, 0:3],
                        sq1[:, :, 0:3], bias["bn1_g"],
                        bias["bn1_b"], epst[:], "bn1", for_timing)
    # b1' = f1_b + f1_w @ t1 (tiny matvec on original f1 tiles)
    b1p = small.tile([128, EC], F32, tag="b1p", name="b1p", bufs=1)
    t1r = small.tile([128, EC], F32R, tag="t1r", name="t1r", bufs=1)
    with nc.allow_low_precision(reason="f32r matvec input"):
        nc.vector.tensor_copy(out=t1r[:], in_=t1[:])
    psb = ps_big.tile([128, 2, NT], F32, tag="sc", name="psb1")
    for m in range(EC):
        for k in range(EC):
            nc.tensor.matmul(psb[:, 0, 2 * m:2 * m + 2],
                             f1[k][:, m * 128:(m + 1) * 128],
                             t1r[:, k:k + 1].to_broadcast((128, 2)),
                             start=(k == 0), stop=(k == EC - 1))
    nc.vector.tensor_tensor(out=b1p[:],
                            in0=psb[:, 0, 0:2 * EC:2], in1=bias["f1"],
                            op=OP.add)
    # fold BN1 scale into f1 (in place, per input-channel partition)
    for k in range(EC):
        with nc.allow_low_precision(reason="f32r weights"):
            nc.vector.tensor_scalar(out=f1[k][:], in0=f1[k][:],
                                    scalar1=s1[:, k:k + 1], scalar2=None,
                                    op0=OP.mult)

    # hn = s1*ht + t1 (BN1 output, residual only) -> xT slots
    hn = [const.tile([128, TOK], F32R, tag=f"xT{k}", name=f"hn{k}")
          for k in range(EC)]
    for m in range(EC):
        for i, (off, fl) in enumerate(_bank_slices(0, TOK)):
            src = ht[m][:, off:off + fl].bitcast(F32)
            dstv = hn[m][:, off:off + fl]
            with nc.allow_low_precision(reason="f32r hn"):
                nc.gpsimd.tensor_scalar(out=dstv, in0=src,
                                        scalar1=s1[:, m:m + 1],
                                        scalar2=t1[:, m:m + 1],
                                        op0=OP.mult, op1=OP.add)

    # ---------- FFN ----------
    h1 = act.tile([128, EC, TOK], F8, tag="h1", name="h1")
    for m in range(EC):
        for off, fl in _bank_slices(0, TOK):
            ps = ps_big.tile([128, 2, NT], F32, tag="sc", name="psf1")
            for k in range(EC):
                nc.tensor.matmul(ps[:, 0, 0:fl], f1[k][:, m * 128:(m + 1) * 128],
                                 ht[k][:, off:off + fl],
                                 start=(k == 0), stop=(k == EC - 1))
            with nc.allow_low_precision(reason="fp8 h1"):
                nc.scalar.activation(out=h1[:, m, off:off + fl],
                                     in_=ps[:, 0, 0:fl], func=AF.Relu,
                                     bias=b1p[:, m:m + 1], scale=1.0)
    ho = [act.tile([128, TOK], F32, tag=f"ht{k}", name=f"ho{k}")
          for k in range(EC)]
    acc2 = small.tile([128, EC, 4], F32, tag="acc2", name="acc2", bufs=1)
    sq2 = small.tile([128, EC, 4], F32, tag="sq2", name="sq2", bufs=1)
    for m in range(EC):
        for i, (off, fl) in enumerate(_bank_slices(0, TOK)):
            ps = ps_big.tile([128, 2, NT], F32, tag="sc", name="psf2")
            for c in range(0, fl, 256):
                cl = min(256, fl - c)
                for g in range(2):
                    nc.tensor.matmul(
                        ps[:, 0, c:c + cl],
                        f28[:, g, :, m * 128:(m + 1) * 128],
                        h1[:, 2 * g:2 * g + 2, off + c:off + c + cl],
                        start=(g == 0), stop=(g == 1), perf_mode=DR)
            dst = ho[m][:, off:off + fl]
            nc.vector.scalar_tensor_tensor(
                out=dst, in0=ps[:, 0, 0:fl], scalar=bias["f2"][:, m:m + 1],
                in1=hn[m][:, off:off + fl].bitcast(F32),
                op0=OP.add, op1=OP.add,
                accum_out=acc2[:, m, i:i + 1])
            # sumsq for BN2 on ACT (DVE is saturated by the STT evacs here)
            scr = small.tile([128, NT], F32, tag="sqa", name="sqa2", bufs=2)
            nc.scalar.activation(
                out=scr[:, 0:fl], in_=dst, func=AF.Square,
                accum_out=sq2[:, m, i:i + 1])

    # ---------- BN2 + output (pipelined per 512-token slice) ----------
    s2, t2 = _bn_params(nc, small, dram, acc2[:], sq2[:],
                        bias["bn2_g"],
                        bias["bn2_b"], epst[:], "bn2", for_timing)
    for i, (off, fl) in enumerate(_bank_slices(0, TOK)):
        for m in range(EC):
            dst = ho[m][:, off:off + fl]
            eng = (i * EC + m) % 3
            if eng == 0:
                nc.vector.tensor_scalar(out=dst, in0=dst,
                                        scalar1=s2[:, m:m + 1],
                                        scalar2=t2[:, m:m + 1],
                                        op0=OP.mult, op1=OP.add)
            elif eng == 1:
                nc.scalar.activation(out=dst, in_=dst, func=AF.Identity,
                                     bias=t2[:, m:m + 1], scale=s2[:, m:m + 1])
            else:
                nc.gpsimd.tensor_scalar(out=dst, in0=dst,
                                        scalar1=s2[:, m:m + 1],
                                        scalar2=t2[:, m:m + 1],
                                        op0=OP.mult, op1=OP.add)
            nc.sync.dma_start(out=yT_d.ap()[m * 128:(m + 1) * 128, off:off + fl],
                              in_=dst)


def _bn_params(nc, small, dram, accs, sqs, g_sb, b_sb, epst, name,
               for_timing=False):
    """Per-channel scale/shift for training-mode BN over all B*N tokens from
    raw per-(m, slice) sums: reduce -> 8-core AllReduce -> mu/var ->
    sqrt+recip. Returns (s [128, EC], t [128, EC]) tiles."""
    ccin = dram.tile([128, 2 * EC], F32, tag=f"cci_{name}", name=f"cci_{name}")
    ccout = dram.tile([128, 2 * EC], F32, tag=f"cco_{name}", name=f"cco_{name}")
    su = small.tile([128, 2, EC], F32, tag=f"su_{name}", name=f"su_{name}")
    gsa = small.tile([128, 2, EC], F32, tag=f"gs_{name}", name=f"gs_{name}")
    nc.vector.tensor_reduce(out=su[:, 0, :], in_=accs,
                            axis=mybir.AxisListType.X, op=OP.add)
    nc.vector.tensor_reduce(out=su[:, 1, :], in_=sqs,
                            axis=mybir.AxisListType.X, op=OP.add)
    nc.sync.dma_start(out=ccin[:], in_=su[:].rearrange("p a b -> p (a b)"))
    if for_timing:
        # TimelineSim cannot model collectives; substitute a same-shape copy
        nc.gpsimd.dma_start(out=ccout[:], in_=ccin[:])
    else:
        nc.gpsimd.collective_compute(
            "AllReduce", OP.add, replica_groups=[list(range(N_CORES))],
            ins=[ccin.opt()], outs=[ccout.opt()])
    nc.sync.dma_start(out=gsa[:].rearrange("p a b -> p (a b)"), in_=ccout[:])
    mu = small.tile([128, EC], F32, tag=f"mu_{name}", name=f"mu_{name}", bufs=1)
    var = small.tile([128, EC], F32, tag=f"var_{name}", name=f"var_{name}",
                     bufs=1)
    nc.vector.tensor_scalar(out=mu[:], in0=gsa[:, 0, :],
                            scalar1=1.0 / N_GLOBAL, scalar2=None, op0=OP.mult)
    nc.vector.tensor_tensor(out=var[:], in0=mu[:], in1=mu[:], op=OP.mult)
    nc.vector.scalar_tensor_tensor(out=var[:], in0=gsa[:, 1, :],
                                   scalar=1.0 / N_GLOBAL, in1=var[:],
                                   op0=OP.mult, op1=OP.subtract)
    sq = small.tile([128, EC], F32, tag=f"sq_{name}", name=f"sq_{name}", bufs=1)
    nc.scalar.activation(out=sq[:], in_=var[:], func=AF.Sqrt, bias=epst,
                         scale=1.0)
    r0 = small.tile([128, EC], F32, tag=f"r0_{name}", name=f"r0_{name}", bufs=1)
    nc.vector.reciprocal(out=r0[:], in_=sq[:])
    s_all = small.tile([128, EC], F32, tag=f"s_{name}", name=f"s_{name}",
                       bufs=1)
    sh_all = small.tile([128, EC], F32, tag=f"sh_{name}", name=f"sh_{name}",
                        bufs=1)
    nc.vector.tensor_tensor(out=s_all[:], in0=r0[:], in1=g_sb, op=OP.mult)
    nc.vector.tensor_tensor(out=sh_all[:], in0=mu[:], in1=s_all[:], op=OP.mult)
    nc.vector.tensor_tensor(out=sh_all[:], in0=b_sb, in1=sh_all[:],
                            op=OP.subtract)
    return s_all, sh_all


_NC_CACHE = None


def _get_nc():
    global _NC_CACHE
    if _NC_CACHE is None:
        _NC_CACHE = build()
    return _NC_CACHE


def make_in_maps(inputs):
    import ml_dtypes
    f8 = ml_dtypes.float8_e4m3
    shared = {}
    for n in W8_NAMES:
        w = np.asarray(inputs[f"{n}_w"], dtype=np.float32)      # [E, E]
        # w8[p, g, jt, j] = W[j, (2g+jt)*128 + p]
        w8 = np.ascontiguousarray(
            w.T.reshape(2, 2, 128, E).transpose(2, 0, 1, 3)).astype(f8)
        shared[f"{n}_w8"] = w8.reshape(128, 4 * E)
    shared["f1_wT"] = np.ascontiguousarray(
        np.asarray(inputs["f1_w"], dtype=np.float32).T)
    w2 = np.asarray(inputs["f2_w"], dtype=np.float32)
    shared["f2_w8"] = np.ascontiguousarray(
        w2.T.reshape(2, 2, 128, E).transpose(2, 0, 1, 3)).astype(
            f8).reshape(128, 4 * E)
    for n in ["rv", "tv"]:
        shared[f"{n}_brep"] = np.ascontiguousarray(
            np.broadcast_to(np.asarray(inputs[f"{n}_b"], dtype=np.float32),
                            (128, E)))
    bpk = np.empty((128, len(ALL_B) * EC), dtype=np.float32)
    for i, n in enumerate(ALL_B):
        vec = inputs[f"{n}_b"] if n in W8_NAMES + ["f1", "f2"] else inputs[n]
        bpk[:, i * EC:(i + 1) * EC] = np.asarray(vec).reshape(EC, 128).T
    shared["bpk"] = bpk
    sel2 = np.zeros((98, 128), dtype=np.float32)
    for p in range(4):
        sel2[32 * p, 0:64] = IVS
        sel2[32 * p + 1, 64:128] = IVS
    shared["sel2"] = sel2
    shared["ones1"] = np.ones((1, 128), dtype=np.float32)

    x = np.asarray(inputs["x"], dtype=np.float32)
    in_maps = []
    for i in range(N_CORES):
        xc = x[BL * i:BL * (i + 1)]                      # [BL, N, E]
        xT = np.ascontiguousarray(xc.transpose(2, 0, 1).reshape(E, TOK))
        x8 = np.ascontiguousarray(
            xT.reshape(EC, 128, BL, N).transpose(1, 0, 2, 3)).astype(f8)
        x8r = np.ascontiguousarray(x8[:, :, :, 0:NR])
        x8t = np.ascontiguousarray(x8[:, :, :, NR:N])
        in_maps.append({"xT": xT,
                        "x8r": x8r.reshape(128, EC * 2 * NR),
                        "x8t": x8t.reshape(128, EC * 2 * NT), **shared})
    return in_maps


def assemble_output(results):
    y = np.empty((B, N, E), dtype=np.float32)
    for i in range(N_CORES):
        yT = results[i]["yT"]                            # [E, TOK]
        y[BL * i:BL * (i + 1)] = yT.reshape(E, BL, N).transpose(1, 2, 0)
    return y


def kernel(**inputs):
    nc = _get_nc()
    in_maps = make_in_maps(inputs)
    res = run_bass_kernel_spmd(nc, in_maps, core_ids=list(range(N_CORES)))
    return assemble_output(res.results)


if __name__ == "__main__":
    nc = build()
    print("build ok")
